# revision 14
# baseline (speedup 1.0000x reference)
"""MemModule (MemAE memory addressing) Bass/Tile kernel for 8x TRN2 NeuronCores.

Reference computation (per token t of B*H*W=16384, C=256, M=2000):
    att = softmax(x_t @ W.T); att = hard_shrink_relu(att, 0.0025); att /= sum(att)
    y_t = att @ W
Outputs: y [16,256,32,32], att [16,2000,32,32].

Numerics (measured on HW):
  - hard_shrink_relu with eps=1e-12 is a pure mask: min |a-lam|/lam over the
    whole dataset is 6.4e-6 >> eps/lam; masked form deviates < 2e-7 abs.
  - mask computed as s > Exp(Ln(S) + ln(lam')): strictly-monotone pushforward
    of the z-space compare z > ln(lam*S); ACT Ln abs err ~3.4e-6 < min gap.
  - fp16 hi/lo 3-pass matmuls give fp32-grade products at 1 cyc/row.
  - lam' carries a +1.79e-6 correction for the measured ACT-exp mean bias.

Sharding: data-parallel over flattened tokens; core i handles batches 2i,2i+1.
"""
import sys, os
import numpy as np

sys.path.insert(0, '/opt/trn_rl_repo')

B, C, H, W = 16, 256, 32, 32
HW = H * W            # 1024 tokens per batch
M = 2000              # memory slots
NCORES = 8
BPC = B // NCORES     # batches per core = 2
NCHUNK = 512          # tokens per chunk
NCH = BPC * HW // NCHUNK   # chunks per core = 4
MP = 2048             # padded memory slots (zero-padded weights; pad slots get
                      # s=exp(0)=1 < threshold so q=0 — exact no-ops downstream)
MT = 128              # m-tile partition size (16 * 128 = 2048)
NMT = MP // MT        # 16
MLAST = M - 15 * MT   # valid rows in last tile = 80
LAM = 0.0025
# measured ACT-exp mean relative bias (-2.08e-6) and ACT-ln bias (+2.9e-7):
# threshold ln(lam*S_true) ~= Ln_act(S_act) + ln(lam) - bias_sum
LN_LAM_ADJ = float(np.log(0.0025) + 1.79e-6)

_CACHE = {}


def _build(repeat=None):
    from contextlib import ExitStack
    import concourse.bass as bass
    import concourse.tile as tile
    from concourse import bacc, mybir

    f32 = mybir.dt.float32
    f16 = mybir.dt.float16
    AF = mybir.ActivationFunctionType
    OP = mybir.AluOpType

    nc = bacc.Bacc("TRN2", target_bir_lowering=False, debug=False, num_devices=1)

    def din(name, shape, dt):
        return nc.dram_tensor(name, list(shape), dt, kind="ExternalInput").ap()

    def dout(name, shape, dt):
        return nc.dram_tensor(name, list(shape), dt, kind="ExternalOutput").ap()

    xh_d = din("xh", (BPC, C, HW), f16)
    xl_d = din("xl", (BPC, C, HW), f16)
    wth_d = din("wth", (C, MP), f16)  # W.T hi (zero-padded to MP)
    wtl_d = din("wtl", (C, MP), f16)  # W.T lo
    w2h_d = din("w2h", (MP, C), f16)  # W hi
    w2l_d = din("w2l", (MP, C), f16)  # W lo
    att_d = dout("att", (BPC, M, HW), f32)
    y_d = dout("y", (BPC, C, HW), f32)

    with tile.TileContext(nc) as tc, ExitStack() as ctx:
        const = ctx.enter_context(tc.tile_pool(name="const", bufs=1))
        wpool = ctx.enter_context(tc.tile_pool(name="w", bufs=1))
        xpool = ctx.enter_context(tc.tile_pool(name="x", bufs=3))
        spool = ctx.enter_context(tc.tile_pool(name="s", bufs=17))
        shpool = ctx.enter_context(tc.tile_pool(name="sh", bufs=17))
        slpool = ctx.enter_context(tc.tile_pool(name="sl", bufs=17))
        qhpool = ctx.enter_context(tc.tile_pool(name="qh", bufs=17))
        qlpool = ctx.enter_context(tc.tile_pool(name="ql", bufs=17))
        gpool = ctx.enter_context(tc.tile_pool(name="g", bufs=4))
        qspool = ctx.enter_context(tc.tile_pool(name="qs", bufs=3))
        apool = ctx.enter_context(tc.tile_pool(name="a", bufs=3))
        bpool = ctx.enter_context(tc.tile_pool(name="b", bufs=2))  # broadcast [128,512] tiles
        ypool = ctx.enter_context(tc.tile_pool(name="y", bufs=2))
        zps = ctx.enter_context(tc.tile_pool(name="zps", bufs=3, space="PSUM"))
        sps = ctx.enter_context(tc.tile_pool(name="sps", bufs=2, space="PSUM"))
        yps = ctx.enter_context(tc.tile_pool(name="yps", bufs=2, space="PSUM"))
        dps = ctx.enter_context(tc.tile_pool(name="dps", bufs=1, space="PSUM"))

        ones = const.tile([MT, 128], f16)
        nc.vector.memset(ones[:], 1.0)
        lnlam = const.tile([128, 1], f32)
        nc.vector.memset(lnlam[:], LN_LAM_ADJ)

        # resident weights
        wth = []
        wtl = []
        for kc in range(2):
            t = wpool.tile([128, MP], f16, tag=f"wth{kc}")
            nc.sync.dma_start(t[:], wth_d[kc * 128:(kc + 1) * 128, :])
            wth.append(t)
            t = wpool.tile([128, MP], f16, tag=f"wtl{kc}")
            nc.sync.dma_start(t[:], wtl_d[kc * 128:(kc + 1) * 128, :])
            wtl.append(t)
        # w2 loads are emitted lazily (first use is mm2, ~halfway into chunk 0)
        # so they don't delay the mm1-critical wth/x DMAs at kernel start.
        w2h = [None] * NMT
        w2l = [None] * NMT

        def load_w2():
            for mt in range(NMT):
                t = wpool.tile([MT, C], f16, tag=f"w2h{mt}", name=f"w2h_t{mt}")
                nc.sync.dma_start(t[:], w2h_d[mt * MT:(mt + 1) * MT, :])
                w2h[mt] = t
                t = wpool.tile([MT, C], f16, tag=f"w2l{mt}", name=f"w2l_t{mt}")
                nc.sync.dma_start(t[:], w2l_d[mt * MT:(mt + 1) * MT, :])
                w2l[mt] = t

        import contextlib
        rep_ctx = tc.For_i(0, repeat, 1) if repeat else contextlib.nullcontext()
        with rep_ctx:
          for chk in range(NCH):
            b = chk // (HW // NCHUNK)
            c0 = (chk % (HW // NCHUNK)) * NCHUNK

            # x chunk tiles (rhs of mm1), hi and lo, per contraction half
            xh = []
            xl = []
            for kc in range(2):
                t = xpool.tile([128, NCHUNK], f16, tag=f"xh{kc}")
                nc.sync.dma_start(t[:], xh_d[b, kc * 128:(kc + 1) * 128, c0:c0 + NCHUNK])
                xh.append(t)
                t = xpool.tile([128, NCHUNK], f16, tag=f"xl{kc}")
                nc.sync.dma_start(t[:], xl_d[b, kc * 128:(kc + 1) * 128, c0:c0 + NCHUNK])
                xl.append(t)

            # ---- pass 1a: z = W.T @ x (fp16 3-pass), exp, fp16 split of s
            s_t = [None] * NMT
            sh_t = [None] * NMT
            sl_t = [None] * NMT
            for mt in range(NMT):
                zp = zps.tile([MT, NCHUNK], f32, tag="z")
                ms = slice(mt * MT, (mt + 1) * MT)
                nc.tensor.matmul(zp[:], wth[0][:, ms], xh[0][:], start=True, stop=False)
                nc.tensor.matmul(zp[:], wth[1][:, ms], xh[1][:], start=False, stop=False)
                nc.tensor.matmul(zp[:], wth[0][:, ms], xl[0][:], start=False, stop=False)
                nc.tensor.matmul(zp[:], wth[1][:, ms], xl[1][:], start=False, stop=False)
                nc.tensor.matmul(zp[:], wtl[0][:, ms], xh[0][:], start=False, stop=False)
                nc.tensor.matmul(zp[:], wtl[1][:, ms], xh[1][:], start=False, stop=True)
                st = spool.tile([MT, NCHUNK], f32, tag="s")
                nc.scalar.activation(st[:], zp[:], AF.Exp)
                sht = shpool.tile([MT, NCHUNK], f16, tag="sh")
                nc.vector.tensor_copy(sht[:], st[:])
                slt = slpool.tile([MT, NCHUNK], f16, tag="sl")
                nc.vector.tensor_tensor(slt[:], st[:], sht[:], op=OP.subtract)
                s_t[mt] = st
                sh_t[mt] = sht
                sl_t[mt] = slt

            # ---- pass 1b: S = sum_m s (broadcast over 128 partitions)
            Sp = sps.tile([128, NCHUNK], f32, tag="S")
            for mt in range(NMT):
                k = MT if mt < NMT - 1 else MLAST
                nc.tensor.matmul(Sp[:], ones[:k, :], sh_t[mt][:k, :], start=(mt == 0), stop=False)
            for mt in range(NMT):
                k = MT if mt < NMT - 1 else MLAST
                nc.tensor.matmul(Sp[:], ones[:k, :], sl_t[mt][:k, :], start=False, stop=(mt == NMT - 1))

            # threshold tile: th = Exp(Ln(S) + ln(lam')) , both on ACT
            lnS = bpool.tile([128, NCHUNK], f32, tag="lnS")
            nc.scalar.activation(lnS[:], Sp[:], AF.Ln)
            th = bpool.tile([128, NCHUNK], f32, tag="th")
            nc.scalar.activation(th[:], lnS[:], AF.Exp, bias=lnlam[:])

            # ---- pass 2a: mask and masked s (fp16 pair)
            qh_t = [None] * NMT
            ql_t = [None] * NMT
            for mt in range(NMT):
                gt = gpool.tile([MT, NCHUNK], f16, tag="g")
                nc.vector.tensor_tensor(gt[:], s_t[mt][:], th[:MT, :], op=OP.is_gt)
                qht = qhpool.tile([MT, NCHUNK], f16, tag="qh")
                nc.vector.tensor_tensor(qht[:], sh_t[mt][:], gt[:], op=OP.mult)
                qlt = qlpool.tile([MT, NCHUNK], f16, tag="ql")
                nc.vector.tensor_tensor(qlt[:], sl_t[mt][:], gt[:], op=OP.mult)
                qh_t[mt] = qht
                ql_t[mt] = qlt

            if chk == 0:
                load_w2()

            # ---- pass 2b: y' = W @ q (fp16 3-pass), accumulated over all m
            yp0 = yps.tile([128, NCHUNK], f32, tag="yp")
            yp1 = yps.tile([128, NCHUNK], f32, tag="yp")
            yp = [yp0, yp1]
            for mt in range(NMT):
                for ct in range(2):
                    cs = slice(ct * 128, (ct + 1) * 128)
                    nc.tensor.matmul(yp[ct][:], w2h[mt][:, cs], qh_t[mt][:],
                                     start=(mt == 0), stop=False)
                    nc.tensor.matmul(yp[ct][:], w2h[mt][:, cs], ql_t[mt][:],
                                     start=False, stop=False)
                    nc.tensor.matmul(yp[ct][:], w2l[mt][:, cs], qh_t[mt][:],
                                     start=False, stop=(mt == NMT - 1))

            # ---- pass 2c: denom = sum_m q (broadcast)
            Dp = dps.tile([128, NCHUNK], f32, tag="D")
            for mt in range(NMT):
                nc.tensor.matmul(Dp[:], ones[:], qh_t[mt][:], start=(mt == 0), stop=False)
            for mt in range(NMT):
                nc.tensor.matmul(Dp[:], ones[:], ql_t[mt][:], start=False, stop=(mt == NMT - 1))

            Dm = bpool.tile([128, NCHUNK], f32, tag="Dm")
            nc.vector.tensor_scalar(Dm[:], Dp[:], 1e-12, None, op0=OP.max)
            rd = bpool.tile([128, NCHUNK], f32, tag="rd")
            nc.vector.reciprocal(rd[:], Dm[:])

            # ---- pass 3: att = q * rd  -> DMA out
            for mt in range(NMT):
                rows = MT if mt < NMT - 1 else MLAST
                qs = qspool.tile([MT, NCHUNK], f32, tag="qs")
                nc.gpsimd.tensor_tensor(qs[:], qh_t[mt][:], ql_t[mt][:], op=OP.add)
                at = apool.tile([MT, NCHUNK], f32, tag="at")
                nc.gpsimd.tensor_tensor(at[:], qs[:], rd[:MT, :], op=OP.mult)
                nc.sync.dma_start(att_d[b, mt * MT:mt * MT + rows, c0:c0 + NCHUNK],
                                  at[:rows, :])

            # ---- y = y' * rd -> DMA out
            for ct in range(2):
                yt = ypool.tile([128, NCHUNK], f32, tag="yt")
                nc.vector.tensor_tensor(yt[:], yp[ct][:], rd[:], op=OP.mult)
                nc.sync.dma_start(y_d[b, ct * 128:(ct + 1) * 128, c0:c0 + NCHUNK], yt[:])

    nc.compile()
    return nc


def _get_nc():
    if "nc" not in _CACHE:
        _CACHE["nc"] = _build()
    return _CACHE["nc"]


def _split16(a):
    hi = a.astype(np.float16)
    lo = (a - hi.astype(np.float32)).astype(np.float16)
    return hi, lo


def kernel(x: np.ndarray, weight: np.ndarray, _trace=False):
    from concourse.bass_utils import run_bass_kernel_spmd

    x = np.ascontiguousarray(np.asarray(x, dtype=np.float32))
    weight = np.ascontiguousarray(np.asarray(weight, dtype=np.float32))
    assert x.shape == (B, C, H, W) and weight.shape == (M, C)

    xf = x.reshape(B, C, HW)
    xh, xl = _split16(xf)
    wpad = np.zeros((MP, C), dtype=np.float32)   # zero-pad memory slots to MP
    wpad[:M] = weight
    wth, wtl = _split16(np.ascontiguousarray(wpad.T))   # [C, MP]
    w2h, w2l = _split16(wpad)                           # [MP, C]

    in_maps = []
    for i in range(NCORES):
        in_maps.append({
            "xh": np.ascontiguousarray(xh[i * BPC:(i + 1) * BPC]),
            "xl": np.ascontiguousarray(xl[i * BPC:(i + 1) * BPC]),
            "wth": wth, "wtl": wtl, "w2h": w2h, "w2l": w2l,
        })

    nc = _get_nc()
    br = run_bass_kernel_spmd(nc, in_maps, list(range(NCORES)), trace=_trace)
    _CACHE["last_result"] = br

    att = np.empty((B, M, HW), dtype=np.float32)
    y = np.empty((B, C, HW), dtype=np.float32)
    for i, r in enumerate(br.results):
        att[i * BPC:(i + 1) * BPC] = r["att"]
        y[i * BPC:(i + 1) * BPC] = r["y"]
    return y.reshape(B, C, H, W), att.reshape(B, M, H, W)


# revision 15
# speedup vs baseline: 1.3098x; 1.3098x over previous
"""MemModule (MemAE memory addressing) Bass/Tile kernel for 8x TRN2 NeuronCores.

Reference computation (per token t of B*H*W=16384, C=256, M=2000):
    att = softmax(x_t @ W.T); att = hard_shrink_relu(att, 0.0025); att /= sum(att)
    y_t = att @ W
Outputs: y [16,256,32,32], att [16,2000,32,32].

Numerics (measured on HW):
  - hard_shrink_relu with eps=1e-12 is a pure mask: min |a-lam|/lam over the
    whole dataset is 6.4e-6 >> eps/lam; masked form deviates < 2e-7 abs.
  - mask computed as s > Exp(Ln(S) + ln(lam')): strictly-monotone pushforward
    of the z-space compare z > ln(lam*S); ACT Ln abs err ~3.4e-6 < min gap.
  - fp16 hi/lo 3-pass matmuls give fp32-grade products at 1 cyc/row.
  - lam' carries a +1.79e-6 correction for the measured ACT-exp mean bias.

Sharding: data-parallel over flattened tokens; core i handles batches 2i,2i+1.
"""
import sys, os
import numpy as np

sys.path.insert(0, '/opt/trn_rl_repo')

B, C, H, W = 16, 256, 32, 32
HW = H * W            # 1024 tokens per batch
M = 2000              # memory slots
NCORES = 8
BPC = B // NCORES     # batches per core = 2
NCHUNK = 512          # tokens per chunk
NCH = BPC * HW // NCHUNK   # chunks per core = 4
MP = 2048             # padded memory slots (zero-padded weights; pad slots get
                      # s=exp(0)=1 < threshold so q=0 — exact no-ops downstream)
MT = 128              # m-tile partition size (16 * 128 = 2048)
NMT = MP // MT        # 16
MLAST = M - 15 * MT   # valid rows in last tile = 80
LAM = 0.0025
# measured ACT-exp mean relative bias (-2.08e-6) and ACT-ln bias (+2.9e-7):
# threshold ln(lam*S_true) ~= Ln_act(S_act) + ln(lam) - bias_sum
LN_LAM_ADJ = float(np.log(0.0025) + 1.79e-6)

_CACHE = {}


def _build(repeat=None):
    from contextlib import ExitStack
    import concourse.bass as bass
    import concourse.tile as tile
    from concourse import bacc, mybir

    f32 = mybir.dt.float32
    f16 = mybir.dt.float16
    AF = mybir.ActivationFunctionType
    OP = mybir.AluOpType

    nc = bacc.Bacc("TRN2", target_bir_lowering=False, debug=False, num_devices=1)

    def din(name, shape, dt):
        return nc.dram_tensor(name, list(shape), dt, kind="ExternalInput").ap()

    def dout(name, shape, dt):
        return nc.dram_tensor(name, list(shape), dt, kind="ExternalOutput").ap()

    xh_d = din("xh", (BPC, C, HW), f16)
    xl_d = din("xl", (BPC, C, HW), f16)
    wth_d = din("wth", (C, MP), f16)  # W.T hi (zero-padded to MP)
    wtl_d = din("wtl", (C, MP), f16)  # W.T lo
    w2h_d = din("w2h", (MP, C), f16)  # W hi
    w2l_d = din("w2l", (MP, C), f16)  # W lo
    att_d = dout("att", (BPC, M, HW), f32)
    y_d = dout("y", (BPC, C, HW), f32)

    with tile.TileContext(nc) as tc, ExitStack() as ctx:
        const = ctx.enter_context(tc.tile_pool(name="const", bufs=1))
        wpool = ctx.enter_context(tc.tile_pool(name="w", bufs=1))
        xpool = ctx.enter_context(tc.tile_pool(name="x", bufs=3))
        spool = ctx.enter_context(tc.tile_pool(name="s", bufs=17))
        shpool = ctx.enter_context(tc.tile_pool(name="sh", bufs=17))
        slpool = ctx.enter_context(tc.tile_pool(name="sl", bufs=17))
        qhpool = ctx.enter_context(tc.tile_pool(name="qh", bufs=17))
        qlpool = ctx.enter_context(tc.tile_pool(name="ql", bufs=17))
        gpool = ctx.enter_context(tc.tile_pool(name="g", bufs=4))
        qspool = ctx.enter_context(tc.tile_pool(name="qs", bufs=3))
        apool = ctx.enter_context(tc.tile_pool(name="a", bufs=3))
        bpool = ctx.enter_context(tc.tile_pool(name="b", bufs=2))  # broadcast [128,512] tiles
        ypool = ctx.enter_context(tc.tile_pool(name="y", bufs=2))
        zps = ctx.enter_context(tc.tile_pool(name="zps", bufs=2, space="PSUM"))
        sps = ctx.enter_context(tc.tile_pool(name="sps", bufs=2, space="PSUM"))
        yps = ctx.enter_context(tc.tile_pool(name="yps", bufs=3, space="PSUM"))
        dps = ctx.enter_context(tc.tile_pool(name="dps", bufs=1, space="PSUM"))

        ones = const.tile([MT, 128], f16)
        nc.vector.memset(ones[:], 1.0)
        lnlam = const.tile([128, 1], f32)
        nc.vector.memset(lnlam[:], LN_LAM_ADJ)

        # resident weights
        wth = []
        wtl = []
        for kc in range(2):
            t = wpool.tile([128, MP], f16, tag=f"wth{kc}")
            nc.sync.dma_start(t[:], wth_d[kc * 128:(kc + 1) * 128, :])
            wth.append(t)
            t = wpool.tile([128, MP], f16, tag=f"wtl{kc}")
            nc.sync.dma_start(t[:], wtl_d[kc * 128:(kc + 1) * 128, :])
            wtl.append(t)
        # w2 loads are emitted lazily (first use is mm2, ~halfway into chunk 0)
        # so they don't delay the mm1-critical wth/x DMAs at kernel start.
        w2h = [None] * NMT
        w2l = [None] * NMT

        def load_w2():
            for mt in range(NMT):
                t = wpool.tile([MT, C], f16, tag=f"w2h{mt}", name=f"w2h_t{mt}")
                nc.sync.dma_start(t[:], w2h_d[mt * MT:(mt + 1) * MT, :])
                w2h[mt] = t
                t = wpool.tile([MT, C], f16, tag=f"w2l{mt}", name=f"w2l_t{mt}")
                nc.sync.dma_start(t[:], w2l_d[mt * MT:(mt + 1) * MT, :])
                w2l[mt] = t

        import contextlib
        rep_ctx = tc.For_i(0, repeat, 1) if repeat else contextlib.nullcontext()
        with rep_ctx:
          for chk in range(NCH):
            b = chk // (HW // NCHUNK)
            c0 = (chk % (HW // NCHUNK)) * NCHUNK

            # x chunk tiles (rhs of mm1), hi and lo, per contraction half
            xh = []
            xl = []
            for kc in range(2):
                t = xpool.tile([128, NCHUNK], f16, tag=f"xh{kc}")
                nc.sync.dma_start(t[:], xh_d[b, kc * 128:(kc + 1) * 128, c0:c0 + NCHUNK])
                xh.append(t)
                t = xpool.tile([128, NCHUNK], f16, tag=f"xl{kc}")
                nc.sync.dma_start(t[:], xl_d[b, kc * 128:(kc + 1) * 128, c0:c0 + NCHUNK])
                xl.append(t)

            # ---- pass 1a: z = W.T @ x (fp16 3-pass), exp, fp16 split of s
            s_t = [None] * NMT
            sh_t = [None] * NMT
            sl_t = [None] * NMT
            for mt in range(NMT):
                zp = zps.tile([MT, NCHUNK], f32, tag="z")
                ms = slice(mt * MT, (mt + 1) * MT)
                nc.tensor.matmul(zp[:], wth[0][:, ms], xh[0][:], start=True, stop=False)
                nc.tensor.matmul(zp[:], wth[1][:, ms], xh[1][:], start=False, stop=False)
                nc.tensor.matmul(zp[:], wth[0][:, ms], xl[0][:], start=False, stop=False)
                nc.tensor.matmul(zp[:], wth[1][:, ms], xl[1][:], start=False, stop=False)
                nc.tensor.matmul(zp[:], wtl[0][:, ms], xh[0][:], start=False, stop=False)
                nc.tensor.matmul(zp[:], wtl[1][:, ms], xh[1][:], start=False, stop=True)
                st = spool.tile([MT, NCHUNK], f32, tag="s")
                nc.scalar.activation(st[:], zp[:], AF.Exp)
                sht = shpool.tile([MT, NCHUNK], f16, tag="sh")
                nc.vector.tensor_copy(sht[:], st[:])
                slt = slpool.tile([MT, NCHUNK], f16, tag="sl")
                nc.vector.tensor_tensor(slt[:], st[:], sht[:], op=OP.subtract)
                s_t[mt] = st
                sh_t[mt] = sht
                sl_t[mt] = slt

            # ---- pass 1b: S = sum_m s (broadcast over 128 partitions)
            Sp = sps.tile([128, NCHUNK], f32, tag="S")
            for mt in range(NMT):
                k = MT if mt < NMT - 1 else MLAST
                nc.tensor.matmul(Sp[:], ones[:k, :], sh_t[mt][:k, :], start=(mt == 0), stop=False)
            for mt in range(NMT):
                k = MT if mt < NMT - 1 else MLAST
                nc.tensor.matmul(Sp[:], ones[:k, :], sl_t[mt][:k, :], start=False, stop=(mt == NMT - 1))

            # threshold tile: th = Exp(Ln(S) + ln(lam')) , both on ACT
            lnS = bpool.tile([128, NCHUNK], f32, tag="lnS")
            nc.scalar.activation(lnS[:], Sp[:], AF.Ln)
            th = bpool.tile([128, NCHUNK], f32, tag="th")
            nc.scalar.activation(th[:], lnS[:], AF.Exp, bias=lnlam[:])

            # ---- pass 2a: mask and masked s (fp16 pair)
            qh_t = [None] * NMT
            ql_t = [None] * NMT
            for mt in range(NMT):
                gt = gpool.tile([MT, NCHUNK], f16, tag="g")
                nc.vector.tensor_tensor(gt[:], s_t[mt][:], th[:MT, :], op=OP.is_gt)
                qht = qhpool.tile([MT, NCHUNK], f16, tag="qh")
                nc.vector.tensor_tensor(qht[:], sh_t[mt][:], gt[:], op=OP.mult)
                qlt = qlpool.tile([MT, NCHUNK], f16, tag="ql")
                nc.vector.tensor_tensor(qlt[:], sl_t[mt][:], gt[:], op=OP.mult)
                qh_t[mt] = qht
                ql_t[mt] = qlt

            if chk == 0:
                load_w2()

            # ---- pass 2b: y' = W @ q (fp16 3-pass), accumulated over all m
            yp0 = yps.tile([128, NCHUNK], f32, tag="yp")
            yp1 = yps.tile([128, NCHUNK], f32, tag="yp")
            yp = [yp0, yp1]
            for mt in range(NMT):
                for ct in range(2):
                    cs = slice(ct * 128, (ct + 1) * 128)
                    nc.tensor.matmul(yp[ct][:], w2h[mt][:, cs], qh_t[mt][:],
                                     start=(mt == 0), stop=False)
                    nc.tensor.matmul(yp[ct][:], w2h[mt][:, cs], ql_t[mt][:],
                                     start=False, stop=False)
                    nc.tensor.matmul(yp[ct][:], w2l[mt][:, cs], qh_t[mt][:],
                                     start=False, stop=(mt == NMT - 1))

            # ---- pass 2c: denom = sum_m q (broadcast)
            Dp = dps.tile([128, NCHUNK], f32, tag="D")
            for mt in range(NMT):
                nc.tensor.matmul(Dp[:], ones[:], qh_t[mt][:], start=(mt == 0), stop=False)
            for mt in range(NMT):
                nc.tensor.matmul(Dp[:], ones[:], ql_t[mt][:], start=False, stop=(mt == NMT - 1))

            Dm = bpool.tile([128, NCHUNK], f32, tag="Dm")
            nc.vector.tensor_scalar(Dm[:], Dp[:], 1e-12, None, op0=OP.max)
            rd = bpool.tile([128, NCHUNK], f32, tag="rd")
            nc.vector.reciprocal(rd[:], Dm[:])

            # ---- pass 3: att = q * rd  -> DMA out
            for mt in range(NMT):
                rows = MT if mt < NMT - 1 else MLAST
                qs = qspool.tile([MT, NCHUNK], f32, tag="qs")
                nc.gpsimd.tensor_tensor(qs[:], qh_t[mt][:], ql_t[mt][:], op=OP.add)
                at = apool.tile([MT, NCHUNK], f32, tag="at")
                nc.gpsimd.tensor_tensor(at[:], qs[:], rd[:MT, :], op=OP.mult)
                nc.sync.dma_start(att_d[b, mt * MT:mt * MT + rows, c0:c0 + NCHUNK],
                                  at[:rows, :])

            # ---- y = y' * rd -> DMA out
            for ct in range(2):
                yt = ypool.tile([128, NCHUNK], f32, tag="yt")
                nc.vector.tensor_tensor(yt[:], yp[ct][:], rd[:], op=OP.mult)
                nc.sync.dma_start(y_d[b, ct * 128:(ct + 1) * 128, c0:c0 + NCHUNK], yt[:])

    nc.compile()
    return nc


def _get_nc():
    if "nc" not in _CACHE:
        _CACHE["nc"] = _build()
    return _CACHE["nc"]


def _split16(a):
    hi = a.astype(np.float16)
    lo = (a - hi.astype(np.float32)).astype(np.float16)
    return hi, lo


def kernel(x: np.ndarray, weight: np.ndarray, _trace=False):
    from concourse.bass_utils import run_bass_kernel_spmd

    x = np.ascontiguousarray(np.asarray(x, dtype=np.float32))
    weight = np.ascontiguousarray(np.asarray(weight, dtype=np.float32))
    assert x.shape == (B, C, H, W) and weight.shape == (M, C)

    xf = x.reshape(B, C, HW)
    xh, xl = _split16(xf)
    wpad = np.zeros((MP, C), dtype=np.float32)   # zero-pad memory slots to MP
    wpad[:M] = weight
    wth, wtl = _split16(np.ascontiguousarray(wpad.T))   # [C, MP]
    w2h, w2l = _split16(wpad)                           # [MP, C]

    in_maps = []
    for i in range(NCORES):
        in_maps.append({
            "xh": np.ascontiguousarray(xh[i * BPC:(i + 1) * BPC]),
            "xl": np.ascontiguousarray(xl[i * BPC:(i + 1) * BPC]),
            "wth": wth, "wtl": wtl, "w2h": w2h, "w2l": w2l,
        })

    nc = _get_nc()
    br = run_bass_kernel_spmd(nc, in_maps, list(range(NCORES)), trace=_trace)
    _CACHE["last_result"] = br

    att = np.empty((B, M, HW), dtype=np.float32)
    y = np.empty((B, C, HW), dtype=np.float32)
    for i, r in enumerate(br.results):
        att[i * BPC:(i + 1) * BPC] = r["att"]
        y[i * BPC:(i + 1) * BPC] = r["y"]
    return y.reshape(B, C, H, W), att.reshape(B, M, H, W)
